# revision 5
# baseline (speedup 1.0000x reference)
"""EvoformerPermuter Trainium2 kernel.

Math (per batch):
  xi  = where(mask, pad, x_in);  xo = x_out + pos
  aff = (xo @ (Wa*diag(w_aff))) @ (xi @ Wb)^T          [512,512]
  E   = exp(aff)   (softmax shifts cancel; b_aff is a constant bias and
                    cancels in both softmaxes, so it is ignored)
  d1  = colsums(E), d2 = rowsums(E)
  K'  = E*diag(1/d1) + diag(1/d2)*E      (= 2*K of the reference; global
                                          scale washes out of Sinkhorn)
  Sinkhorn in diagonal-scaling form, T iterations:
      u = 1/(E(v/d1) + (E v)/d2)         [uses ET tiles]
      v = 1/(ET(u/d2) + (ET u)/d1)       [uses E tiles]
  P   = diag(u) K' diag(v)
      = E .* (u (x) (v/d1) + (u/d2) (x) v)    -- exactly column-stochastic,
        matching the reference's final col-normalize at convergence.

T=14 suffices: the iterate reaches the fp32 fixed point (vs the
reference's fixed 20 iterations the difference is ~3.7e-6, at the
reference's own fp32 noise floor).

Sharding: data-parallel over batch, 8 batches per core x 8 cores.

Layouts on device (per core, NB=8 batches):
  E  [128, b, ci, 512] : E[p, b, ci, j]  = E_b[128*ci+p, j]   (i on partitions)
  ET [128, b, cj, 512] : ET[p, b, cj, i] = E_b[i, 128*cj+p]   (j on partitions)
  vectors in "W" form [128, 64]: col (c*8+b)*2 + k, k=0 scaled-vec, k=1 raw
  per half-step: 4 accumulating f32r matvec MMs (M=2) -> psum [2,512]
  -> ACT/DVE copy -> 4 PE transposes [2,128]->[128,2] -> psumT [128,64]
  -> DVE math (reciprocal etc.) -> next W  (all f32r streams; psum fp32)
"""
import numpy as np
from contextlib import ExitStack

import concourse.bacc as bacc
import concourse.tile as tile
import concourse.mybir as mybir
from concourse.masks import make_identity
from concourse.bass_utils import run_bass_kernel_spmd

F32 = mybir.dt.float32
F32R = mybir.dt.float32r
U8 = mybir.dt.uint8
EXP = mybir.ActivationFunctionType.Exp

B, N, D, EDIM = 64, 512, 256, 128
NCORES = 8
NB = B // NCORES          # batches per core
C = N // 128              # partition chunks per matrix dim
DC = D // 128             # d-dim chunks
T_ITERS = 14

_CACHE = {}


def _build():
    nc = bacc.Bacc()
    x_in = nc.dram_tensor("x_in", [NB, N, D], F32, kind="ExternalInput")
    x_out = nc.dram_tensor("x_out", [NB, N, D], F32, kind="ExternalInput")
    maskp = nc.dram_tensor("maskp", [NB, 128, C], U8, kind="ExternalInput")
    wa = nc.dram_tensor("wa", [D, EDIM], F32, kind="ExternalInput")
    wb = nc.dram_tensor("wb", [D, EDIM], F32, kind="ExternalInput")
    poswat = nc.dram_tensor("poswat", [EDIM, N], F32, kind="ExternalInput")
    pad = nc.dram_tensor("pad", [1, D], F32, kind="ExternalInput")
    out = nc.dram_tensor("out", [NB, N, N], F32, kind="ExternalOutput")

    with tile.TileContext(nc) as tc, ExitStack() as ctx:
        ctx.enter_context(nc.allow_low_precision(
            reason="f32r vectors: 1.2e-4 rounding is within the Sinkhorn noise budget"))
        res = ctx.enter_context(tc.tile_pool(name="res", bufs=1))

        ident = res.tile([128, 128], F32)
        make_identity(nc, ident)

        sb_wa = res.tile([128, DC, EDIM], F32R)
        sb_wb = res.tile([128, DC, EDIM], F32R)
        sb_poswat = res.tile([128, N], F32)
        sb_pad = res.tile([128, D], F32)
        nc.sync.dma_start(sb_wa, wa[:, :].rearrange("(c p) e -> p c e", p=128).bitcast(F32R))
        nc.sync.dma_start(sb_wb, wb[:, :].rearrange("(c p) e -> p c e", p=128).bitcast(F32R))
        nc.sync.dma_start(sb_poswat, poswat[:, :])
        nc.sync.dma_start(sb_pad, pad[:, :].to_broadcast((128, D)))

        sb_E = res.tile([128, NB, C, N], F32R)
        sb_ET = res.tile([128, NB, C, N], F32R)
        d1 = res.tile([128, NB, C], F32)
        d2 = res.tile([128, NB, C], F32)

        # ---------------- setup phase ----------------
        with tc.tile_pool(name="sps", bufs=2, space="PSUM") as sps, \
             tc.tile_pool(name="sx", bufs=2) as sx, \
             tc.tile_pool(name="sy", bufs=1) as sy:
            for b in range(NB):
                xin_t = sx.tile([128, C, D], F32, tag="xin")
                xout_t = sx.tile([128, C, D], F32, tag="xout")
                m8 = sx.tile([128, C], U8, tag="m8")
                nc.sync.dma_start(xin_t, x_in[b].rearrange("(c p) d -> p c d", p=128))
                nc.sync.dma_start(xout_t, x_out[b].rearrange("(c p) d -> p c d", p=128))
                nc.sync.dma_start(m8, maskp[b])

                xi = sy.tile([128, C, D], F32, tag="xi")
                for c in range(C):
                    nc.vector.select(xi[:, c, :], m8[:, c : c + 1].to_broadcast((128, D)),
                                     sb_pad, xin_t[:, c, :])

                xiT = sy.tile([128, DC, N], F32R, tag="xiT")
                xoT = sy.tile([128, DC, N], F32R, tag="xoT")
                for src, dstT in ((xi, xiT), (xout_t, xoT)):
                    for dc in range(DC):
                        pst = sps.tile([128, N], F32, tag="tx")
                        for c in range(C):
                            nc.tensor.transpose(pst[:, 128 * c : 128 * (c + 1)],
                                                src[:, c, 128 * dc : 128 * (dc + 1)],
                                                ident)
                        nc.vector.tensor_copy(dstT[:, dc, :], pst)

                psA = sps.tile([128, N], F32, tag="pa")
                psB = sps.tile([128, N], F32, tag="pa")
                for dc in range(DC):
                    nc.tensor.matmul(psA, sb_wa[:, dc, :], xoT[:, dc, :],
                                     start=(dc == 0), stop=(dc == DC - 1))
                for dc in range(DC):
                    nc.tensor.matmul(psB, sb_wb[:, dc, :], xiT[:, dc, :],
                                     start=(dc == 0), stop=(dc == DC - 1))
                aT = sy.tile([128, N], F32R, tag="aT")
                bT = sy.tile([128, N], F32R, tag="bT")
                # aT = psA + poswat  (pos folded into the a-projection)
                nc.vector.scalar_tensor_tensor(aT, psA, 1.0, sb_poswat,
                                               mybir.AluOpType.mult,
                                               mybir.AluOpType.add)
                nc.scalar.copy(bT, psB)

                for ci in range(C):
                    psF = sps.tile([128, N], F32, tag="pf")
                    nc.tensor.matmul(psF, aT[:, 128 * ci : 128 * (ci + 1)], bT,
                                     start=True, stop=True)
                    nc.scalar.activation(sb_E[:, b, ci, :], psF, EXP,
                                         accum_out=d2[:, b, ci : ci + 1])
                for cj in range(C):
                    psF = sps.tile([128, N], F32, tag="pf")
                    nc.tensor.matmul(psF, bT[:, 128 * cj : 128 * (cj + 1)], aT,
                                     start=True, stop=True)
                    nc.scalar.activation(sb_ET[:, b, cj, :], psF, EXP,
                                         accum_out=d1[:, b, cj : cj + 1])

        # iteration-layout inverse-marginal tensors: cols x = c*NB + b
        invd1W = res.tile([128, C * NB], F32)
        invd2W = res.tile([128, C * NB], F32)
        nc.vector.reciprocal(invd1W.rearrange("p (c b) -> p b c", b=NB), d1)
        nc.vector.reciprocal(invd2W.rearrange("p (c b) -> p b c", b=NB), d2)

        fs = res.tile([128, C, 4 * NB], F32)   # final stage: cols 4*b + kind

        # ---------------- Sinkhorn iterations ----------------
        with tc.tile_pool(name="mv", bufs=4, space="PSUM") as mvp, \
             tc.tile_pool(name="pt", bufs=2, space="PSUM") as ptp, \
             tc.tile_pool(name="wp", bufs=2) as wp, \
             tc.tile_pool(name="cpp", bufs=4) as cpp, \
             tc.tile_pool(name="mp", bufs=2) as mp:

            w_cur = wp.tile([128, C * NB * 2], F32R, tag="W")
            # init: v = ones -> cols k=0 hold invd1 (v/d1), k=1 hold ones
            wv0 = w_cur.rearrange("p (x k) -> p x k", k=2)
            ones = mp.tile([128, C * NB], F32, tag="ones")
            nc.vector.memset(ones, 1.0)
            nc.vector.tensor_copy(wv0[:, :, 1], ones)
            nc.vector.tensor_copy(wv0[:, :, 0], invd1W)

            for t in range(T_ITERS):
                for half in range(2):   # 0: u-step (uses ET), 1: v-step (uses E)
                    rhs_all = sb_ET if half == 0 else sb_E
                    d_here = invd2W if half == 0 else invd1W

                    psumT = ptp.tile([128, C * NB * 2], F32, tag="pt")
                    for b in range(NB):
                        mv = mvp.tile([2, N], F32, tag="mv")
                        for c in range(C):
                            nc.tensor.matmul(
                                mv, w_cur[:, (c * NB + b) * 2 : (c * NB + b) * 2 + 2],
                                rhs_all[:, b, c, :],
                                start=(c == 0), stop=(c == C - 1))
                        cp = cpp.tile([2, N], F32, tag="cp")
                        if b % 2 == 0:
                            nc.scalar.copy(cp, mv)
                        else:
                            nc.vector.tensor_copy(cp, mv)
                        for c in range(C):
                            nc.tensor.transpose(
                                psumT[:, (c * NB + b) * 2 : (c * NB + b) * 2 + 2],
                                cp[:, 128 * c : 128 * (c + 1)], ident[:2, :2])

                    vT = psumT.rearrange("p (x k) -> p x k", k=2)
                    w_next = wp.tile([128, C * NB * 2], F32R, tag="W")
                    wv = w_next.rearrange("p (x k) -> p x k", k=2)
                    tmp = mp.tile([128, C * NB], F32, tag="tmp")
                    ssum = mp.tile([128, C * NB], F32, tag="ssum")
                    nc.vector.tensor_mul(tmp, vT[:, :, 1], d_here)
                    nc.vector.tensor_add(ssum, tmp, vT[:, :, 0])
                    nc.vector.reciprocal(wv[:, :, 1], ssum)
                    nc.vector.tensor_mul(wv[:, :, 0], wv[:, :, 1].bitcast(F32), d_here)

                    if t == T_ITERS - 1:
                        # stash (u, u/d2) resp. (v/d1, v) for the final pass
                        fv = fs.rearrange("p c (b k) -> p c b k", k=4)
                        wn = w_next.rearrange("p (c b k) -> p c b k", b=NB, k=2)
                        if half == 0:
                            nc.vector.tensor_copy(fv[:, :, :, 0], wn[:, :, :, 1].bitcast(F32))
                            nc.vector.tensor_copy(fv[:, :, :, 1], wn[:, :, :, 0].bitcast(F32))
                        else:
                            nc.vector.tensor_copy(fv[:, :, :, 2], wn[:, :, :, 0].bitcast(F32))
                            nc.vector.tensor_copy(fv[:, :, :, 3], wn[:, :, :, 1].bitcast(F32))
                    w_cur = w_next

        # ---------------- final: P = E .* (U V^T) ----------------
        with tc.tile_pool(name="fps", bufs=1, space="PSUM") as fps, \
             tc.tile_pool(name="gps", bufs=3, space="PSUM") as gps, \
             tc.tile_pool(name="fsb", bufs=1) as fsb, \
             tc.tile_pool(name="pout", bufs=4) as pout:

            psR = fps.tile([32, N], F32)
            for c in range(C):
                nc.tensor.transpose(psR[:, 128 * c : 128 * (c + 1)],
                                    fs[:, c, :], ident)
            frows = fsb.tile([32, N], F32)
            nc.scalar.copy(frows, psR)

            for b in range(NB):
                fu = fsb.tile([2, N], F32R, tag=f"fu{b}")
                fv_ = fsb.tile([2, N], F32R, tag=f"fv{b}")
                nc.sync.dma_start(fu, frows[4 * b : 4 * b + 2, :].bitcast(F32R))
                nc.sync.dma_start(fv_, frows[4 * b + 2 : 4 * b + 4, :].bitcast(F32R))
                for ci in range(C):
                    psG = gps.tile([128, N], F32, tag="pg")
                    nc.tensor.matmul(psG, fu[:, 128 * ci : 128 * (ci + 1)], fv_,
                                     start=True, stop=True)
                    p_t = pout.tile([128, N], F32, tag="p")
                    nc.vector.tensor_mul(p_t, sb_E[:, b, ci, :].bitcast(F32), psG)
                    nc.sync.dma_start(
                        out[b].rearrange("(c p) n -> p c n", p=128)[:, ci, :], p_t)

    nc.finalize()
    return nc


def kernel(node_embeddings_inputs, node_masks_inputs, node_embeddings_outputs,
           node_padding_features, positional_encoding_outputs,
           W_a, W_b, w_aff, b_aff):
    # b_aff is a constant bias on aff; softmax(x + const) == softmax(x) along
    # both axes, so it cancels exactly and is ignored.
    x_in = np.ascontiguousarray(np.asarray(node_embeddings_inputs, dtype=np.float32))
    x_out = np.ascontiguousarray(np.asarray(node_embeddings_outputs, dtype=np.float32))
    mask = np.asarray(node_masks_inputs)
    pad_f = np.asarray(node_padding_features, dtype=np.float32).reshape(1, D)
    pos = np.asarray(positional_encoding_outputs, dtype=np.float32).reshape(N, D)
    wa_f = np.asarray(W_a, dtype=np.float32) * np.asarray(w_aff, dtype=np.float32)[None, :]
    wb_f = np.ascontiguousarray(np.asarray(W_b, dtype=np.float32))
    poswat_f = np.ascontiguousarray((pos @ wa_f).T)       # [E, N]
    wa_f = np.ascontiguousarray(wa_f)
    # mask in [b, p, c] layout with i = c*128 + p
    maskp = np.ascontiguousarray(
        mask.reshape(B, C, 128).transpose(0, 2, 1)).astype(np.uint8)

    if "nc" not in _CACHE:
        _CACHE["nc"] = _build()
    nc = _CACHE["nc"]

    in_maps = []
    for core in range(NCORES):
        sl = slice(core * NB, (core + 1) * NB)
        in_maps.append(dict(
            x_in=x_in[sl], x_out=x_out[sl], maskp=maskp[sl],
            wa=wa_f, wb=wb_f, poswat=poswat_f, pad=pad_f,
        ))
    res = run_bass_kernel_spmd(nc, in_maps, list(range(NCORES)))
    return np.concatenate([r["out"] for r in res.results], axis=0)
